# revision 6
# baseline (speedup 1.0000x reference)
"""Trainium2 Bass kernel: NeptuneTransformerEncoderLayer on 8 NeuronCores.

Sharding: batch(4) x seq-half(2) -> 8 cores, zero collectives.  Each core
computes K/V for its batch's full 2048 tokens and Q/attention/FFN for its
own 1024 tokens (host permutes src so own queries are rows [0:1024)).

v2: fp8e4 DoubleRow matmuls (0.5 cyc/row) for QKV, PV, out-proj and FFN
down-proj; f16 for scores and FFN up-proj (precision).  Weights are
host-side pre-transposed/pre-scaled/pre-quantized (no on-device weight
transposes).  The ACT-bound attention is split into two 512-query blocks
and the second block's attention is interleaved with the first block's
PE-bound FFN so both engines stay busy.  Scores PSUM tiles pair kc chunks
so exp keeps 1024-wide slices; PV emission is skewed one kc-pair behind
scores so the PE never waits on exp.
"""
import sys

for _p in ("/opt/trn_rl_repo", "/root/.axon_site/_ro/trn_rl_repo"):
    if _p not in sys.path:
        sys.path.insert(0, _p)

import numpy as np
import ml_dtypes

import concourse.bass as bass
import concourse.mybir as mybir
import concourse.tile as tile
from concourse import bacc
from concourse import bass_utils

F8 = mybir.dt.float8e4
F16 = mybir.dt.float16
F32 = mybir.dt.float32
AF = mybir.ActivationFunctionType
DR = mybir.MatmulPerfMode.DoubleRow
NP8 = ml_dtypes.float8_e4m3

P = 128
D = 1024            # d_model
DC = D // P         # 8 d-model chunks
NH = 16             # heads
HD = 64             # head dim
DFF = 4096
FC = DFF // P       # 32 ff chunks
S = 2048            # full sequence per batch
TQ = 1024           # query tokens per core
QB = 512            # query pipeline block
N_CORES = 8
EPS = 1e-5
BASE = 10000.0

SX = 8.0            # x_norm fp8 scale
SW = 64.0           # fp8 weight scale
SV = 8.0            # v fp8 scale
SH = 8.0            # hidden fp8 scale (via w3)
EXP_BIAS = 1.0      # exp(0.125*s - EXP_BIAS); score stats: max 4.36, rowmax>=1.2


def build_nc():
    nc = bacc.Bacc("TRN2", target_bir_lowering=False, debug=False,
                   num_devices=N_CORES)
    src = nc.dram_tensor("src", [S, D], F32, kind="ExternalInput")
    cos_t = nc.dram_tensor("cos_t", [P, S], F16, kind="ExternalInput")
    sin_t = nc.dram_tensor("sin_t", [P, S], F16, kind="ExternalInput")
    wqkv8 = nc.dram_tensor("wqkv8", [P, 4, 2, 3 * D], F8, kind="ExternalInput")
    wo8 = nc.dram_tensor("wo8", [P, 4, 2, D], F8, kind="ExternalInput")
    w13t = nc.dram_tensor("w13t", [P, DC, 2, DFF], F16, kind="ExternalInput")
    w28 = nc.dram_tensor("w28", [P, FC // 2, 2, D], F8, kind="ExternalInput")
    out = nc.dram_tensor("out", [TQ, D], F32, kind="ExternalOutput")

    with tile.TileContext(nc) as tc:
        emit(nc, tc, src, cos_t, sin_t, wqkv8, wo8, w13t, w28, out)
    nc.compile()
    return nc


def emit(nc, tc, src, cos_t, sin_t, wqkv8, wo8, w13t, w28, out):
    from contextlib import ExitStack

    ctx = ExitStack()
    with ctx:
        g_xnt = ctx.enter_context(ExitStack())   # XNT8/C2/S2/WQ8, freed post-B
        persist = ctx.enter_context(tc.tile_pool(name="persist", bufs=1))
        p_att = ctx.enter_context(tc.tile_pool(name="p_att", bufs=1))
        p_xnt = g_xnt.enter_context(
            tc.tile_pool(name="p_xnt", bufs=1, side="right"))
        p_qkv = ctx.enter_context(tc.tile_pool(name="p_qkv", bufs=1))

        XNT8 = p_xnt.tile([P, DC, S], F8)        # x_norm1.T fp8 (x8)
        WQ8 = p_xnt.tile([P, 4, 2, 3 * D], F8)   # qkv weights (24KB/p)
        C2 = p_xnt.tile([P, S], F16)
        S2 = p_xnt.tile([P, S], F16)
        QT = p_qkv.tile([P, DC, TQ], F16)        # roped q.T (true scale)
        KT = p_qkv.tile([P, DC, S], F16)         # roped k.T
        VA = p_qkv.tile([P, S // P, NH * 65], F8)   # v8 + ones col per head
        ATT = p_att.tile([P, DC, TQ], F8)        # attn out .T, fp8 (x8)
        WO8b = persist.tile([P, 4, 2, D], F8)
        eps_t = persist.tile([P, 1], F32)
        nc.vector.memset(eps_t[:], EPS)
        eps64_t = persist.tile([P, 1], F32)
        nc.vector.memset(eps64_t[:], EPS / (SX * SX))
        nbias_t = persist.tile([P, 1], F32)
        nc.vector.memset(nbias_t[:], -EXP_BIAS)

        nc.sync.dma_start(C2[:], cos_t[:])
        nc.sync.dma_start(S2[:], sin_t[:])
        nc.scalar.dma_start(WQ8[:], wqkv8[:])
        nc.scalar.dma_start(WO8b[:], wo8[:])

        va3 = VA.rearrange("p t (h c) -> p t h c", c=65)

        # long-lived working pools first (LIFO stack: closed last)
        phc = ctx.enter_context(tc.tile_pool(name="phc", bufs=2))
        phn = ctx.enter_context(tc.tile_pool(name="phn", bufs=1))
        pha_q = ctx.enter_context(tc.tile_pool(name="pha_q", bufs=1))
        phd = ctx.enter_context(tc.tile_pool(name="phd", bufs=2))
        phd_s = ctx.enter_context(tc.tile_pool(name="phd_s", bufs=4))
        phe_w = ctx.enter_context(tc.tile_pool(name="phe_w", bufs=3))
        phe = ctx.enter_context(tc.tile_pool(name="phe", bufs=2))
        phf = ctx.enter_context(tc.tile_pool(name="phf", bufs=2))
        phf_w = ctx.enter_context(tc.tile_pool(name="phf_w", bufs=2))
        # A/B working pools on top of the stack: freed before the overlap
        g_ab = ctx.enter_context(ExitStack())
        pha = g_ab.enter_context(tc.tile_pool(name="pha", bufs=2))
        pha_x = g_ab.enter_context(tc.tile_pool(name="pha_x", bufs=3))
        pha_t = g_ab.enter_context(tc.tile_pool(name="pha_t", bufs=2))
        pha_s = g_ab.enter_context(tc.tile_pool(name="pha_s", bufs=4))
        phb = g_ab.enter_context(tc.tile_pool(name="phb", bufs=2))

        st4 = [None]
        w13p = [None]

        def emit_a_ti(ti, _unused):
            if ti % 4 == 0:
                st4[0] = pha.tile([P, 4, D], F16, tag="src_in", name="st4")
                nc.gpsimd.dma_start(
                    st4[0][:], src.ap()[ti * P:(ti + 4) * P, :].rearrange(
                        "(g p) d -> p g d", g=4))
            st = st4[0][:, ti % 4, :]
            sqd = pha_q.tile([P, D], F16, tag="sqd")
            ssq = pha_s.tile([P, 1], F32, tag="ssq")
            nc.vector.scalar_tensor_tensor(
                sqd[:], st, 1.0, st, op0=mybir.AluOpType.mult,
                op1=mybir.AluOpType.mult, accum_out=ssq[:])
            # rms/SX = sqrt(ssq/(D*SX^2) + eps/SX^2) -> rinv = SX/rms
            rms = pha_s.tile([P, 1], F32, tag="rms")
            nc.scalar.activation(rms[:], ssq[:], AF.Sqrt,
                                 bias=eps64_t[:], scale=1.0 / (D * SX * SX))
            rinv = pha_s.tile([P, 1], F32, tag="rinv")
            nc.vector.reciprocal(rinv[:], rms[:])
            xn = pha_x.tile([P, D], F16, tag="xn")
            nc.vector.tensor_scalar_mul(xn[:], st, rinv[:])
            t16 = pha_t.tile([P, DC, P], F16, tag="t16")
            nc.sync.dma_start(t16[:], xn[:], transpose=True)
            nc.scalar.activation(XNT8[:, :, ti * P:(ti + 1) * P], t16[:],
                                 AF.Copy)

        def qk_slice(j, ts, ps_work):
            # DoubleRow qkv projection for 512 tokens of q (j<8) / k
            sl = slice(ts * 512, ts * 512 + 512)
            pk = ps_work.tile([P, 512], F32, tag="work")
            for c2 in range(4):
                nc.tensor.matmul(pk[:], WQ8[:, c2, :, j * P:(j + 1) * P],
                                 XNT8[:, 2 * c2:2 * c2 + 2, sl],
                                 start=(c2 == 0), stop=(c2 == 3),
                                 perf_mode=DR)
            # rope: evict to f16 (true scale), rotate pairs
            pk16 = phb.tile([P, 512], F16, tag="pk16")
            nc.vector.tensor_scalar_mul(pk16[:], pk[:], 1.0 / (SX * SW))
            aa = phb.tile([P, 512], F16, tag="aa")
            nc.gpsimd.tensor_mul(aa[:], pk16[:], C2[:, sl])
            pp = phb.tile([P, 512], F16, tag="pp")
            nc.vector.tensor_mul(pp[:], pk16[:], S2[:, sl])
            bb = phb.tile([P, 512], F16, tag="bb")
            for h0 in (0, 64):
                nc.vector.tensor_copy(bb[h0:h0 + 32, :],
                                      pp[h0 + 32:h0 + 64, :])
                nc.vector.tensor_copy(bb[h0 + 32:h0 + 64, :],
                                      pp[h0:h0 + 32, :])
            dst = (QT[:, j, sl] if j < 8 else KT[:, j - 8, sl])
            nc.vector.tensor_add(dst, aa[:], bb[:])

        def emit_v_ti(ti, ps_work):
            for half in range(2):
                pvh = ps_work.tile([P, 512], F32, tag="work")
                for j4 in range(4):
                    j = 16 + half * 4 + j4
                    for c2 in range(4):
                        nc.tensor.matmul(
                            pvh[:, j4 * P:(j4 + 1) * P],
                            XNT8[:, 2 * c2:2 * c2 + 2, ti * P:(ti + 1) * P],
                            WQ8[:, c2, :, j * P:(j + 1) * P],
                            start=(c2 == 0), stop=(c2 == 3),
                            perf_mode=DR)
                hs = slice(half * 8, (half + 1) * 8)
                nc.vector.memset(va3[:, ti, hs, 64], 1.0)
                nc.vector.tensor_scalar_mul(
                    va3[:, ti, hs, 0:64],
                    pvh.rearrange("p (h c) -> p h c", c=64),
                    SV / (SX * SW))

        def attend(h, qb, ps_st, ps_pv, part=2):
            """Attention for head h, query block qb.  Yields after each
            emission chunk so the caller can interleave other engine work
            (part = number of chunks)."""
            j, hb = h // 2, 64 * (h % 2)
            qsl = slice(qb * QB, (qb + 1) * QB)
            ppv = ps_pv.tile([65, QB], F32, tag="ppv")
            pend = None   # skew: PV for pair kcp emitted after scores kcp+1
            for kcp in range(8):
                pst = ps_st.tile([P, 2, QB], F32, tag="pst")
                for e in range(2):
                    kc = 2 * kcp + e
                    nc.tensor.matmul(
                        pst[:, e, :],
                        KT[hb:hb + 64, j, kc * P:(kc + 1) * P],
                        QT[hb:hb + 64, j, qsl],
                        start=True, stop=True)
                pt8 = phc.tile([P, 2, QB], F8, tag="pt8")
                nc.scalar.activation(pt8[:], pst[:], AF.Exp,
                                     scale=0.125, bias=nbias_t[:])
                if pend is not None:
                    pk, pt = pend
                    nc.tensor.matmul(ppv[:],
                                     VA[:, 2 * pk:2 * pk + 2,
                                        65 * h:65 * h + 65],
                                     pt[:], start=(pk == 0), stop=False,
                                     perf_mode=DR)
                pend = (kcp, pt8)
                if part == 2 and kcp == 3:
                    yield
            pk, pt = pend
            nc.tensor.matmul(ppv[:],
                             VA[:, 2 * pk:2 * pk + 2, 65 * h:65 * h + 65],
                             pt[:], start=False, stop=True, perf_mode=DR)
            rec = phn.tile([1, QB], F32, tag="rec")
            nc.vector.reciprocal(rec[:], ppv[64:65, :])
            nrmS = phn.tile([64, QB], F32, tag="nrmS")
            nc.gpsimd.partition_broadcast(nrmS[:], rec[:])
            nc.vector.tensor_mul(ATT[hb:hb + 64, j, qsl], ppv[0:64, :],
                                 nrmS[:])
            yield

        def run_all(gen):
            for _ in gen:
                pass

        def emit_d_tb(tb, phd_ps, _unused):
            py = phd_ps.tile([P, D], F32, tag="py")
            for j in range(8):
                for c2 in range(4):
                    nc.tensor.matmul(py[:, j * P:(j + 1) * P],
                                     ATT[:, 2 * c2:2 * c2 + 2,
                                         tb * P:(tb + 1) * P],
                                     WO8b[:, c2, :, j * P:(j + 1) * P],
                                     start=(c2 == 0), stop=(c2 == 3),
                                     perf_mode=DR)
            srcq = phd.tile([P, D], F16, tag="srcq")
            nc.gpsimd.dma_start(srcq[:], src[tb * P:(tb + 1) * P, :])
            nc.vector.scalar_tensor_tensor(
                X2[:, tb, :], py[:], 1.0 / (SV * SW), srcq[:],
                op0=mybir.AluOpType.mult, op1=mybir.AluOpType.add)
            sqd2 = pha_q.tile([P, D], F16, tag="sqd")
            ssq = phd_s.tile([P, 1], F32, tag="ssq")
            nc.vector.scalar_tensor_tensor(
                sqd2[:], X2[:, tb, :], 1.0, X2[:, tb, :],
                op0=mybir.AluOpType.mult, op1=mybir.AluOpType.mult,
                accum_out=ssq[:])
            rms = phd_s.tile([P, 1], F32, tag="rms")
            nc.scalar.activation(rms[:], ssq[:], AF.Sqrt,
                                 bias=eps_t[:], scale=1.0 / D)
            rinv = phd_s.tile([P, 1], F32, tag="rinv")
            nc.vector.reciprocal(rinv[:], rms[:])
            xn2 = phd.tile([P, D], F16, tag="xn2")
            nc.vector.tensor_scalar_mul(xn2[:], X2[:, tb, :], rinv[:])
            nc.sync.dma_start(XN2T[:, :, tb * P:(tb + 1) * P], xn2[:],
                              transpose=True)

        def emit_e_fc(fc, qb, phe_ps, dge=None):
            qsl = slice(qb * QB, (qb + 1) * QB)
            w13c = phe_w.tile([P, DC, 2, P], F16, tag="w13c")
            (dge or nc.sync).dma_start(w13c[:],
                                       w13t[:, :, :, fc * P:(fc + 1) * P])
            pab = phe_ps.tile([P, 2, QB], F32, tag="pab")
            for s in range(2):
                for c in range(DC):
                    nc.tensor.matmul(pab[:, s, :], w13c[:, c, s, :],
                                     XN2T[:, c, qsl],
                                     start=(c == 0), stop=(c == DC - 1))
            # silu via tanh (same ACT table as Exp -> no table reloads):
            # 2*silu(x) = x*(1+tanh(x/2)); the 1/2 is folded into w3t.
            th = phe.tile([P, QB], F16, tag="th")
            nc.scalar.activation(th[:], pab[:, 0, :], AF.Tanh, scale=0.5)
            sa = phe.tile([P, QB], F16, tag="sa")
            nc.vector.scalar_tensor_tensor(
                sa[:], th[:], 1.0, pab[:, 0, :],
                op0=mybir.AluOpType.add, op1=mybir.AluOpType.mult)
            # HT8 holds one query block at a time (reused across qb passes)
            nc.vector.tensor_mul(HT8[:, fc, :], sa[:], pab[:, 1, :])

        def emit_e_fc_h(fc, phe_ps):
            # E for query block 0 in two 256-wide passes (1-bank PSUM tiles)
            for half in range(2):
                qsl = slice(half * 256, half * 256 + 256)
                w13c = phe_w.tile([P, DC, 2, P], F16, tag="w13c")
                if half == 0:
                    nc.sync.dma_start(w13c[:],
                                      w13t[:, :, :, fc * P:(fc + 1) * P])
                    w13k = w13c
                else:
                    w13k = w13p[0]
                w13p[0] = w13k
                pab = phe_ps.tile([P, 2, 256], F32, tag="pab")
                for s in range(2):
                    for c in range(DC):
                        nc.tensor.matmul(pab[:, s, :], w13k[:, c, s, :],
                                         XN2T[:, c, qsl],
                                         start=(c == 0), stop=(c == DC - 1))
                th = phe.tile([P, 256], F16, tag="th")
                nc.scalar.activation(th[:], pab[:, 0, :], AF.Tanh, scale=0.5)
                sa = phe.tile([P, 256], F16, tag="sa")
                nc.vector.scalar_tensor_tensor(
                    sa[:], th[:], 1.0, pab[:, 0, :],
                    op0=mybir.AluOpType.add, op1=mybir.AluOpType.mult)
                nc.vector.tensor_mul(HT8[:, fc, qsl], sa[:], pab[:, 1, :])

        def emit_f_qb(qb, phf_ps):
            # token-major w2 matmul: lhsT = hidden (stationary), rhs = w2
            # (moving) -> psum [tokens, features]; no transpose fold needed.
            pzts = [phf_ps.tile([P, D], F32, tag=f"pzt{tq}", name=f"pzt{tq}")
                    for tq in range(QB // P)]
            for j in range(8):
                w2j = phf_w.tile([P, FC // 2, 2, P], F8, tag="w2j")
                nc.sync.dma_start(w2j[:],
                                  w28[:, :, :, j * P:(j + 1) * P])
                for tq in range(QB // P):
                    for fp in range(FC // 2):
                        nc.tensor.matmul(
                            pzts[tq][:, j * P:(j + 1) * P],
                            HT8[:, 2 * fp:2 * fp + 2, tq * P:(tq + 1) * P],
                            w2j[:, fp, :, :],
                            start=(fp == 0), stop=(fp == FC // 2 - 1),
                            perf_mode=DR)
            for tq in range(QB // P):
                tb = qb * (QB // P) + tq
                nc.vector.scalar_tensor_tensor(
                    X2[:, tb, :], pzts[tq][:], 1.0 / (SH * SW), X2[:, tb, :],
                    op0=mybir.AluOpType.mult, op1=mybir.AluOpType.add)
                nc.sync.dma_start(out[tb * P:(tb + 1) * P, :], X2[:, tb, :])

        # ---------------- A + B + C(qb0), woven ----------------
        with tc.tile_pool(name="ps_work", bufs=2, space="PSUM") as ps_work, \
             tc.tile_pool(name="ps_st", bufs=2, space="PSUM") as ps_st, \
             tc.tile_pool(name="ps_pv", bufs=2, space="PSUM") as ps_pv:
            for ti in range(4):
                emit_a_ti(ti, None)
            emit_v_ti(0, ps_work)
            qk_slice(0, 0, ps_work)
            for ti in range(4, 8):
                emit_a_ti(ti, None)
                emit_v_ti(ti - 3, ps_work)
            qk_slice(0, 1, ps_work)
            qk_slice(8, 0, ps_work)
            for ti in range(8, 12):
                emit_a_ti(ti, None)
                emit_v_ti(ti - 3, ps_work)
            qk_slice(8, 1, ps_work)
            qk_slice(8, 2, ps_work)
            for ti in range(12, 16):
                emit_a_ti(ti, None)
                emit_v_ti(ti - 3, ps_work)
            qk_slice(8, 3, ps_work)
            for ti in range(13, S // P):
                emit_v_ti(ti, ps_work)
            run_all(attend(0, 0, ps_st, ps_pv))
            run_all(attend(1, 0, ps_st, ps_pv))
            for hp in range(1, 8):
                for jj in (hp, 8 + hp):
                    for ts in range(2 if jj < 8 else 4):
                        qk_slice(jj, ts, ps_work)
                run_all(attend(2 * hp, 0, ps_st, ps_pv))
                run_all(attend(2 * hp + 1, 0, ps_st, ps_pv))
        g_xnt.close()   # free XNT8 / WQ8 / C2 / S2
        g_ab.close()    # free A/B working pools

        # D/E/F big tiles go where XNT8/WQ8 were
        p_mid = ctx.enter_context(tc.tile_pool(name="p_mid", bufs=1,
                                               side="right"))
        X2 = p_mid.tile([P, TQ // P, D], F32)    # residual+output (t-major)
        XN2T = p_mid.tile([P, DC, TQ], F16)      # x_norm2.T
        HT8 = p_mid.tile([P, FC, QB], F8)        # swiglu hidden .T (x8)

        # ---------------- D(qb0) ----------------
        with tc.tile_pool(name="phd_ps", bufs=4, space="PSUM") as phd_ps:
            for tb in range(4):
                emit_d_tb(tb, phd_ps, None)

        # ---------- overlap: C(qb1) interleaved with E(qb0) ----------
        with tc.tile_pool(name="ps_st2", bufs=2, space="PSUM") as ps_st2, \
             tc.tile_pool(name="ps_pv2", bufs=2, space="PSUM") as ps_pv2, \
             tc.tile_pool(name="phe_ps", bufs=1, space="PSUM") as phe_ps:
            for h in range(NH):
                gen = attend(h, 1, ps_st2, ps_pv2)
                next(gen)
                emit_e_fc(2 * h, 0, phe_ps)
                run_all(gen)
                emit_e_fc(2 * h + 1, 0, phe_ps)
        # p_qkv (QT/KT/VA) stays allocated; fits alongside p_mid

        # ---------------- D(qb1), then F(qb0) hiding D1's latency ------
        with tc.tile_pool(name="phd_ps", bufs=4, space="PSUM") as phd_ps:
            for tb in range(4, 8):
                emit_d_tb(tb, phd_ps, None)
        with tc.tile_pool(name="phf_ps", bufs=1, space="PSUM") as phf_ps:
            emit_f_qb(0, phf_ps)

        # ---------------- E(qb1) + F(qb1) ----------------
        with tc.tile_pool(name="phe_ps2", bufs=3, space="PSUM") as phe_ps2:
            for fc in range(FC):
                emit_e_fc(fc, 1, phe_ps2)
        with tc.tile_pool(name="phf_ps", bufs=1, space="PSUM") as phf_ps:
            emit_f_qb(1, phf_ps)


_NC_CACHE = None


def _get_nc():
    global _NC_CACHE
    if _NC_CACHE is None:
        _NC_CACHE = build_nc()
    return _NC_CACHE


def _host_tables(positions_b, axis_scale):
    """Build parity-split fp16 cos/sin tables (128, S) for one batch."""
    coord = positions_b * axis_scale[None, :]              # (S, 4)
    invf = BASE ** (-(np.arange(0, 16, 2, dtype=np.float32) / 16.0))  # (8,)
    ang = coord[:, :, None] * invf[None, None, :]          # (S, 4, 8)
    ang = ang.reshape(S, 32).T                             # (32, S): r = 8a + j
    cos64 = np.concatenate([np.cos(ang), np.cos(ang)], axis=0)   # (64, S)
    sin64 = np.concatenate([np.sin(ang), -np.sin(ang)], axis=0)  # (64, S)
    c128 = np.concatenate([cos64, cos64], axis=0).astype(np.float16)
    s128 = np.concatenate([sin64, sin64], axis=0).astype(np.float16)
    return c128, s128


def _prep_weights(inputs):
    """Pre-transpose / permute / scale / quantize all weights on the host."""
    n1 = np.asarray(inputs["norm1_w"], np.float32)
    n2 = np.asarray(inputs["norm2_w"], np.float32)
    w_qkv = np.asarray(inputs["w_qkv"], np.float32) * n1[None, :]
    w_out = np.asarray(inputs["w_out"], np.float32)
    w1 = np.asarray(inputs["w1"], np.float32) * n2[None, :]
    w3 = np.asarray(inputs["w3"], np.float32) * n2[None, :]
    w2 = np.asarray(inputs["w2"], np.float32)

    # qkv row permutation: for j<16 (q,k) parity split rows; v plain
    rows = np.zeros(3 * D, np.int64)
    for j in range(24):
        for col in range(P):
            if j < 16:
                h, r = divmod(col, 64)
                par, jp = divmod(r, 32)
                rows[j * P + col] = 128 * j + 64 * h + 2 * jp + par
            else:
                rows[j * P + col] = j * P + col
    wq_perm = w_qkv[rows, :]                   # (3D, D) row-permuted
    # wqkv8[p, c2, e, jf]  = wq_perm[jf, (2*c2+e)*128 + p] * SW
    wqkv8 = (wq_perm.T.reshape(4, 2, P, 3 * D).transpose(2, 0, 1, 3)
             * SW).astype(NP8)
    wqkv8 = np.ascontiguousarray(wqkv8)
    # wo8[p, c2, e, f] = w_out[f, (2*c2+e)*128 + p] * SW
    wo8 = (w_out.T.reshape(4, 2, P, D).transpose(2, 0, 1, 3) * SW).astype(NP8)
    wo8 = np.ascontiguousarray(wo8)
    # w13t[p, c, s, f]: s=0 -> w1[f, c*128+p], s=1 -> w3[f, c*128+p]*SH/2
    w1tt = w1.T.reshape(DC, P, DFF).transpose(1, 0, 2)
    w3tt = w3.T.reshape(DC, P, DFF).transpose(1, 0, 2) * (SH / 2)
    w13t = np.ascontiguousarray(
        np.stack([w1tt, w3tt], axis=2)).astype(np.float16)
    # w28[p, fp, e, f] = w2[f, (2*fp+e)*128 + p] * SW
    w28 = (w2.T.reshape(FC // 2, 2, P, D).transpose(2, 0, 1, 3)
           * SW).astype(NP8)
    w28 = np.ascontiguousarray(w28)
    return {"wqkv8": wqkv8, "wo8": wo8, "w13t": w13t, "w28": w28}


def build_in_maps(inputs):
    src = np.asarray(inputs["src"], dtype=np.float32)
    positions = np.asarray(inputs["positions"], dtype=np.float32)
    axis_scale = np.asarray(inputs["axis_scale"], np.float32)
    weights = _prep_weights(inputs)
    in_maps = []
    for c in range(N_CORES):
        b, h = c // 2, c % 2
        sp = src[b]
        pp = positions[b]
        if h == 1:  # own half first
            sp = np.concatenate([sp[TQ:], sp[:TQ]], axis=0)
            pp = np.concatenate([pp[TQ:], pp[:TQ]], axis=0)
        ct, st = _host_tables(pp, axis_scale)
        m = {"src": np.ascontiguousarray(sp), "cos_t": ct, "sin_t": st}
        m.update(weights)
        in_maps.append(m)
    return in_maps


def kernel(src, positions, w_qkv, w_out, norm1_w, norm2_w, w1, w2, w3,
           axis_scale):
    src = np.asarray(src, dtype=np.float32)
    B = src.shape[0]
    in_maps = build_in_maps(dict(
        src=src, positions=positions, w_qkv=w_qkv, w_out=w_out,
        norm1_w=norm1_w, norm2_w=norm2_w, w1=w1, w2=w2, w3=w3,
        axis_scale=axis_scale))
    nc = _get_nc()
    res = bass_utils.run_bass_kernel_spmd(nc, in_maps,
                                          core_ids=list(range(N_CORES)))
    outp = np.zeros((B, S, D), np.float32)
    for c in range(N_CORES):
        b, h = c // 2, c % 2
        outp[b, h * TQ:(h + 1) * TQ, :] = res.results[c]["out"]
    return outp
